# revision 16
# baseline (speedup 1.0000x reference)
"""Trainium2 Bass kernel for causal multi-head attention (GPT2-ish block).

Full-input contract: kernel(**inputs) takes the unsharded inputs of
  x: [4, 2048, 768], W_Q/W_K/W_V: [12, 768, 64], W_O: [12, 64, 768],
  b_Q/b_K/b_V: [12, 64], b_O: [768]
and returns out: [4, 2048, 768] fp32.

Sharding: 8 cores = 4 batches x 2 head-groups (6 heads each); each core
computes its batch's attention for its 6 heads through W_O (partial sum over
heads). Host sums the two head-group partials per batch and adds b_O.

Per-core layouts (all matmuls are out = lhsT.T @ rhs on the PE):
  xT   [768, 2048]      (d_model on partitions)
  QT,KT[384, 2048]      (head-features on partitions)  q pre-scaled by 1/8
  V    [2048, 384]      (+ ones column per head for softmax denominators)
  scoresT tiles [sk=128, sq<=512] -> exp on ACT -> causal zero on GPSIMD
  zT'  [65, 512]        rows 0-63 unnormalized z, row 64 = softmax denom
  y    [2048, 768] fp32

Structure notes:
 - Heads are processed in pairs on partition rows 0:64 / 64:128 of one
   feature tile; their K=64 scores matmuls go to different PE row groups and
   run concurrently in the array (measured dstart ~4ns).
 - Projections are emitted interleaved with attention chunks so the PE never
   idles long enough for the HAM clock gate to re-throttle.
 - Scores psum tiles come in [128, 2, 512] two-bank groups so a kt-pair is
   exp'd by a single ACTIVATE (amortizes the ~350-cycle ACT overhead).
 - Diagonal tiles are trimmed to their valid sq suffix (512-c columns).
"""

import numpy as np
import ml_dtypes

BF16 = ml_dtypes.bfloat16

S = 2048
D = 768
NH = 12
H = 64
P = 128
NH_LOC = 6
FEAT = NH_LOC * H          # 384
KO = D // P                # 6 contraction tiles for the projections
FT = FEAT // P             # 3 feature partition-tiles
CH = 512                   # sq chunk width
NCH = S // CH              # 4
ST = S // P                # 16 sequence partition-tiles
KPC = CH // P              # 4 sk-tiles per chunk
N_CORES = 8

_NC_CACHE = {}


def _build_nc():
    import concourse.bass as bass
    import concourse.mybir as mybir
    import concourse.tile as tile
    from concourse import bacc

    f32 = mybir.dt.float32
    bf16 = mybir.dt.bfloat16
    AF = mybir.ActivationFunctionType
    ALU = mybir.AluOpType

    nc = bacc.Bacc("TRN2", target_bir_lowering=False, debug=False)

    xT = nc.dram_tensor("xT", [D, S], bf16, kind="ExternalInput").ap()
    wqkv = nc.dram_tensor("wqkv", [D, 3 * FEAT], bf16, kind="ExternalInput").ap()
    bqkv = nc.dram_tensor("bqkv", [3 * FEAT], f32, kind="ExternalInput").ap()
    wo = nc.dram_tensor("wo", [FEAT, D], bf16, kind="ExternalInput").ap()
    y = nc.dram_tensor("y", [S, D], f32, kind="ExternalOutput").ap()

    with tile.TileContext(nc) as tc:
        with (
            tc.tile_pool(name="persist", bufs=1) as persist,
            tc.tile_pool(name="mm_ps", bufs=2, space="PSUM") as mm_ps,
            tc.tile_pool(name="sc_ps", bufs=3, space="PSUM") as sc_ps,
            tc.tile_pool(name="z_ps", bufs=3, space="PSUM") as z_ps,
            tc.tile_pool(name="p_pool", bufs=6) as p_pool,
            tc.tile_pool(name="n_pool", bufs=4) as n_pool,
            tc.tile_pool(name="y_pool", bufs=3) as y_pool,
        ):
            # ---- load inputs into SBUF ----
            # ordered so the first projection chunk can start ASAP: weights
            # and x chunk-0 land first, interleaved per contraction tile
            x_sb = persist.tile([P, KO, S], bf16)
            xT_t = xT.rearrange("(ko pi) s -> pi ko s", pi=P)
            w_sb = persist.tile([P, KO, 3 * FEAT], bf16)
            w_t = wqkv.rearrange("(ko pi) f -> pi ko f", pi=P)
            for ko in range(KO):
                nc.sync.dma_start(out=w_sb[:, ko, :], in_=w_t[:, ko, :])
                nc.sync.dma_start(out=x_sb[:, ko, 0:CH], in_=xT_t[:, ko, 0:CH])
            for c in range(1, NCH):
                for ko in range(KO):
                    nc.sync.dma_start(
                        out=x_sb[:, ko, c * CH : (c + 1) * CH],
                        in_=xT_t[:, ko, c * CH : (c + 1) * CH],
                    )

            bias_sb = persist.tile([P, 9], f32)
            nc.sync.dma_start(out=bias_sb, in_=bqkv.rearrange("(t pi) -> pi t", pi=P))

            bvb_sb = persist.tile([P, FEAT], f32)
            bv_slice = bqkv[2 * FEAT : 3 * FEAT]
            bv_bcast = bass.AP(
                tensor=bv_slice.tensor,
                offset=bv_slice.offset,
                ap=[[0, P]] + list(bv_slice.ap),
            )
            nc.sync.dma_start(out=bvb_sb, in_=bv_bcast)

            wo_sb = persist.tile([P, FT, D], bf16)
            wo_t = wo.rearrange("(ft pi) d -> pi ft d", pi=P)
            nc.sync.dma_start(out=wo_sb, in_=wo_t)

            qT_sb = persist.tile([P, FT, S], bf16)
            kT_sb = persist.tile([P, FT, S], bf16)
            v_sb = persist.tile([P, ST, NH_LOC, H + 1], bf16)
            zn_sb = persist.tile([P, FT, S], bf16)

            # ones column per head for softmax denominators
            nc.vector.memset(v_sb[:, :, :, H : H + 1], 1.0)

            # 0/1 causal mask in the trimmed diagonal frame: keep f >= p
            mask_sb = persist.tile([P, CH], bf16)
            nc.gpsimd.memset(mask_sb, 1.0)
            nc.gpsimd.affine_select(
                out=mask_sb,
                in_=mask_sb,
                compare_op=ALU.is_ge,
                fill=0.0,
                base=0,
                pattern=[[1, CH]],
                channel_multiplier=-1,
            )

            # preload the exp table on ACT so the first real exp doesn't pay
            # the ~2.7us ACT_TABLE_LOAD in the middle of the pipeline
            warm = n_pool.tile([1, 1], f32, tag="warm")
            nc.vector.memset(warm, 0.0)
            nc.scalar.activation(out=warm, in_=warm, func=AF.Exp)

            def proj_chunk(c):
                """QT/KT chunk c and V tiles for sequence chunk c."""
                for which, base in ((0, 0), (1, FEAT)):
                    sb = qT_sb if which == 0 else kT_sb
                    for ft in range(FT):
                        ps = mm_ps.tile([P, CH], f32, tag="mm", name="proj_ps")
                        for ko in range(KO):
                            nc.tensor.matmul(
                                ps,
                                lhsT=w_sb[:, ko, base + ft * P : base + (ft + 1) * P],
                                rhs=x_sb[:, ko, c * CH : (c + 1) * CH],
                                start=(ko == 0),
                                stop=(ko == KO - 1),
                            )
                        nc.vector.tensor_scalar_add(
                            out=sb[:, ft, c * CH : (c + 1) * CH],
                            in0=ps,
                            scalar1=bias_sb[:, which * FT + ft : which * FT + ft + 1],
                        )
                for sti in range(KPC):
                    st = c * KPC + sti
                    ps = mm_ps.tile([P, FEAT], f32, tag="mm", name="projv_ps")
                    for ko in range(KO):
                        nc.tensor.matmul(
                            ps,
                            lhsT=x_sb[:, ko, st * P : (st + 1) * P],
                            rhs=w_sb[:, ko, 2 * FEAT : 3 * FEAT],
                            start=(ko == 0),
                            stop=(ko == KO - 1),
                        )
                    nc.vector.tensor_add(
                        out=v_sb[:, st, :, 0:H],
                        in0=ps.rearrange("p (n h) -> p n h", h=H),
                        in1=bvb_sb.rearrange("p (n h) -> p n h", h=H),
                    )

            def attn_chunk(q):
                nkt = (q + 1) * KPC
                for hp in range(NH_LOC // 2):
                    ft = hp
                    h0, h1 = 2 * hp, 2 * hp + 1
                    zts = [
                        z_ps.tile([H + 1, CH], f32, tag="z", name="zt0"),
                        z_ps.tile([H + 1, CH], f32, tag="z", name="zt1"),
                    ]
                    for kt in range(nkt):
                        c = max(0, kt * P - q * CH)
                        pss = []
                        for hi, r0 in ((0, 0), (1, H)):
                            ps = sc_ps.tile([P, CH], f32, tag="sc", name="sc_ps_t")
                            nc.tensor.matmul(
                                ps[:, c:],
                                lhsT=kT_sb[r0 : r0 + H, ft, kt * P : (kt + 1) * P],
                                rhs=qT_sb[r0 : r0 + H, ft, q * CH + c : (q + 1) * CH],
                                start=True,
                                stop=True,
                            )
                            pss.append(ps)
                        pts = []
                        for hi in (0, 1):
                            pt = p_pool.tile([P, CH], bf16, tag="pt", name="pt_t")
                            nc.scalar.activation(
                                out=pt[:, c:], in_=pss[hi][:, c:], func=AF.Exp
                            )
                            if kt >= q * KPC:
                                # zero entries with sk > sq; in the trimmed
                                # frame keep f' >= p -> multiply by mask prefix
                                nc.vector.tensor_mul(
                                    out=pt[:, c:],
                                    in0=pt[:, c:],
                                    in1=mask_sb[:, 0 : CH - c],
                                )
                            pts.append(pt)
                        for hi, h in ((0, h0), (1, h1)):
                            nc.tensor.matmul(
                                zts[hi][:, c:],
                                lhsT=v_sb[:, kt, h, :],
                                rhs=pts[hi][:, c:],
                                start=(kt == 0),
                                stop=(kt == nkt - 1),
                            )
                    # normalize rows 0..63 by row 64 (the denominator)
                    for hi, r0 in ((0, 0), (1, H)):
                        zt = zts[hi]
                        # custom-DVE ops read partition 0 regardless of the
                        # AP's base partition — stage the denominator row there
                        drow = n_pool.tile([1, CH], f32, tag="drow", name="drow_t")
                        nc.vector.tensor_copy(out=drow, in_=zt[H : H + 1, :])
                        rrow = n_pool.tile([1, CH], f32, tag="rrow", name="rrow_t")
                        nc.vector.reciprocal_approx_fast(out=rrow, in_=drow)
                        rb = n_pool.tile([H, CH], f32, tag="rb", name="rb_t")
                        nc.gpsimd.partition_broadcast(rb, rrow)
                        nc.vector.tensor_mul(
                            out=zn_sb[r0 : r0 + H, ft, q * CH : (q + 1) * CH],
                            in0=zt[0:H, :],
                            in1=rb,
                        )

            def out_chunk(q):
                # output projection for this chunk's sequence tiles;
                # dh pairs share the stationary zn tile per ft
                for sti in range(KPC):
                    st = q * KPC + sti
                    ysb = y_pool.tile([P, D], f32, tag="y", name="y_t")
                    psa = mm_ps.tile([P, D // 2], f32, tag="mm", name="yps_a")
                    psb = mm_ps.tile([P, D // 2], f32, tag="mm", name="yps_b")
                    for ft in range(FT):
                        for ps, dh in ((psa, 0), (psb, 1)):
                            nc.tensor.matmul(
                                ps,
                                lhsT=zn_sb[:, ft, st * P : (st + 1) * P],
                                rhs=wo_sb[:, ft, dh * (D // 2) : (dh + 1) * (D // 2)],
                                start=(ft == 0),
                                stop=(ft == FT - 1),
                            )
                    for ps, dh in ((psa, 0), (psb, 1)):
                        nc.vector.tensor_copy(
                            out=ysb[:, dh * (D // 2) : (dh + 1) * (D // 2)], in_=ps
                        )
                    nc.sync.dma_start(out=y[st * P : (st + 1) * P, :], in_=ysb)

            # interleave: projections for chunk c feed attention chunk c;
            # proj(c+1) is emitted before out(c) so the PE has independent
            # work while the last head pair of chunk c normalizes
            proj_chunk(0)
            for c in range(NCH):
                attn_chunk(c)
                if c + 1 < NCH:
                    proj_chunk(c + 1)
                out_chunk(c)

    nc.compile()
    return nc


def _get_nc():
    if "nc" not in _NC_CACHE:
        _NC_CACHE["nc"] = _build_nc()
    return _NC_CACHE["nc"]


def _shard_inputs(x, W_Q, W_K, W_V, W_O, b_Q, b_K, b_V):
    """Build the 8 per-core input maps. Core c -> (batch c % 4, head-group c // 4)."""
    in_maps = []
    scale = np.float32(1.0 / np.sqrt(H))
    for c in range(N_CORES):
        b = c % 4
        g = c // 4
        hs = slice(g * NH_LOC, (g + 1) * NH_LOC)
        xTb = np.ascontiguousarray(x[b].T).astype(BF16)
        wq = (W_Q[hs].transpose(1, 0, 2).reshape(D, FEAT) * scale).astype(BF16)
        wk = W_K[hs].transpose(1, 0, 2).reshape(D, FEAT).astype(BF16)
        wv = W_V[hs].transpose(1, 0, 2).reshape(D, FEAT).astype(BF16)
        wqkv = np.ascontiguousarray(np.concatenate([wq, wk, wv], axis=1))
        bqkv = np.concatenate(
            [
                (b_Q[hs].reshape(FEAT) * scale).astype(np.float32),
                b_K[hs].reshape(FEAT).astype(np.float32),
                b_V[hs].reshape(FEAT).astype(np.float32),
            ]
        )
        wob = np.ascontiguousarray(W_O[hs].reshape(FEAT, D)).astype(BF16)
        in_maps.append({"xT": xTb, "wqkv": wqkv, "bqkv": bqkv, "wo": wob})
    return in_maps


def _run(in_maps, trace=False):
    from concourse.bass_utils import run_bass_kernel_spmd

    nc = _get_nc()
    return run_bass_kernel_spmd(nc, in_maps, core_ids=list(range(N_CORES)), trace=trace)


def kernel(
    normalized_resid_pre,
    W_Q,
    W_K,
    W_V,
    W_O,
    b_Q,
    b_K,
    b_V,
    b_O,
):
    x = np.asarray(normalized_resid_pre, dtype=np.float32)
    in_maps = _shard_inputs(
        x,
        np.asarray(W_Q, np.float32),
        np.asarray(W_K, np.float32),
        np.asarray(W_V, np.float32),
        np.asarray(W_O, np.float32),
        np.asarray(b_Q, np.float32),
        np.asarray(b_K, np.float32),
        np.asarray(b_V, np.float32),
    )
    res = _run(in_maps)
    bO = np.asarray(b_O, np.float32)
    out = np.empty((4, S, D), dtype=np.float32)
    for b in range(4):
        out[b] = res.results[b]["y"] + res.results[4 + b]["y"] + bO
    return out


# revision 17
# speedup vs baseline: 1.0686x; 1.0686x over previous
"""Trainium2 Bass kernel for causal multi-head attention (GPT2-ish block).

Full-input contract: kernel(**inputs) takes the unsharded inputs of
  x: [4, 2048, 768], W_Q/W_K/W_V: [12, 768, 64], W_O: [12, 64, 768],
  b_Q/b_K/b_V: [12, 64], b_O: [768]
and returns out: [4, 2048, 768] fp32.

Sharding: 8 cores = 4 batches x 2 head-groups (6 heads each); each core
computes its batch's attention for its 6 heads through W_O (partial sum over
heads). Host sums the two head-group partials per batch and adds b_O.

Per-core layouts (all matmuls are out = lhsT.T @ rhs on the PE):
  xT   [768, 2048]      (d_model on partitions)
  QT,KT[384, 2048]      (head-features on partitions)  q pre-scaled by 1/8
  V    [2048, 384]      (+ ones column per head for softmax denominators)
  scoresT tiles [sk=128, sq<=512] -> exp on ACT -> causal zero on GPSIMD
  zT'  [65, 512]        rows 0-63 unnormalized z, row 64 = softmax denom
  y    [2048, 768] fp32

Structure notes:
 - Heads are processed in pairs on partition rows 0:64 / 64:128 of one
   feature tile; their K=64 scores matmuls go to different PE row groups and
   run concurrently in the array (measured dstart ~4ns).
 - Projections are emitted interleaved with attention chunks so the PE never
   idles long enough for the HAM clock gate to re-throttle.
 - Scores psum tiles come in [128, 2, 512] two-bank groups so a kt-pair is
   exp'd by a single ACTIVATE (amortizes the ~350-cycle ACT overhead).
 - Diagonal tiles are trimmed to their valid sq suffix (512-c columns).
"""

import numpy as np
import ml_dtypes

BF16 = ml_dtypes.bfloat16

S = 2048
D = 768
NH = 12
H = 64
P = 128
NH_LOC = 6
FEAT = NH_LOC * H          # 384
KO = D // P                # 6 contraction tiles for the projections
FT = FEAT // P             # 3 feature partition-tiles
CH = 512                   # sq chunk width
NCH = S // CH              # 4
ST = S // P                # 16 sequence partition-tiles
KPC = CH // P              # 4 sk-tiles per chunk
N_CORES = 8

_NC_CACHE = {}


def _build_nc():
    import concourse.bass as bass
    import concourse.mybir as mybir
    import concourse.tile as tile
    from concourse import bacc

    f32 = mybir.dt.float32
    bf16 = mybir.dt.bfloat16
    AF = mybir.ActivationFunctionType
    ALU = mybir.AluOpType

    nc = bacc.Bacc("TRN2", target_bir_lowering=False, debug=False)

    xT = nc.dram_tensor("xT", [D, S], bf16, kind="ExternalInput").ap()
    wqkv = nc.dram_tensor("wqkv", [D, 3 * FEAT], bf16, kind="ExternalInput").ap()
    bqkv = nc.dram_tensor("bqkv", [3 * FEAT], f32, kind="ExternalInput").ap()
    wo = nc.dram_tensor("wo", [FEAT, D], bf16, kind="ExternalInput").ap()
    y = nc.dram_tensor("y", [S, D], f32, kind="ExternalOutput").ap()

    with tile.TileContext(nc) as tc:
        with (
            tc.tile_pool(name="persist", bufs=1) as persist,
            tc.tile_pool(name="mm_ps", bufs=2, space="PSUM") as mm_ps,
            tc.tile_pool(name="sc_ps", bufs=3, space="PSUM") as sc_ps,
            tc.tile_pool(name="z_ps", bufs=3, space="PSUM") as z_ps,
            tc.tile_pool(name="p_pool", bufs=6) as p_pool,
            tc.tile_pool(name="n_pool", bufs=4) as n_pool,
            tc.tile_pool(name="y_pool", bufs=3) as y_pool,
        ):
            # ---- load inputs into SBUF ----
            # ordered so the first projection chunk can start ASAP: weights
            # and x chunk-0 land first, interleaved per contraction tile
            x_sb = persist.tile([P, KO, S], bf16)
            xT_t = xT.rearrange("(ko pi) s -> pi ko s", pi=P)
            w_sb = persist.tile([P, KO, 3 * FEAT], bf16)
            w_t = wqkv.rearrange("(ko pi) f -> pi ko f", pi=P)
            for ko in range(KO):
                nc.sync.dma_start(out=w_sb[:, ko, :], in_=w_t[:, ko, :])
                nc.sync.dma_start(out=x_sb[:, ko, 0:CH], in_=xT_t[:, ko, 0:CH])
            for c in range(1, NCH):
                for ko in range(KO):
                    nc.sync.dma_start(
                        out=x_sb[:, ko, c * CH : (c + 1) * CH],
                        in_=xT_t[:, ko, c * CH : (c + 1) * CH],
                    )

            bias_sb = persist.tile([P, 9], f32)
            nc.sync.dma_start(out=bias_sb, in_=bqkv.rearrange("(t pi) -> pi t", pi=P))

            bvb_sb = persist.tile([P, FEAT], f32)
            bv_slice = bqkv[2 * FEAT : 3 * FEAT]
            bv_bcast = bass.AP(
                tensor=bv_slice.tensor,
                offset=bv_slice.offset,
                ap=[[0, P]] + list(bv_slice.ap),
            )
            nc.sync.dma_start(out=bvb_sb, in_=bv_bcast)

            wo_sb = persist.tile([P, FT, D], bf16)
            wo_t = wo.rearrange("(ft pi) d -> pi ft d", pi=P)
            nc.sync.dma_start(out=wo_sb, in_=wo_t)

            qT_sb = persist.tile([P, FT, S], bf16)
            kT_sb = persist.tile([P, FT, S], bf16)
            v_sb = persist.tile([P, ST, NH_LOC, H + 1], bf16)
            zn_sb = persist.tile([P, FT, S], bf16)

            # ones column per head for softmax denominators
            nc.vector.memset(v_sb[:, :, :, H : H + 1], 1.0)

            # 0/1 causal mask in the trimmed diagonal frame: keep f >= p
            mask_sb = persist.tile([P, CH], bf16)
            nc.gpsimd.memset(mask_sb, 1.0)
            nc.gpsimd.affine_select(
                out=mask_sb,
                in_=mask_sb,
                compare_op=ALU.is_ge,
                fill=0.0,
                base=0,
                pattern=[[1, CH]],
                channel_multiplier=-1,
            )

            # preload the exp table on ACT so the first real exp doesn't pay
            # the ~2.7us ACT_TABLE_LOAD in the middle of the pipeline
            warm = n_pool.tile([1, 1], f32, tag="warm")
            nc.vector.memset(warm, 0.0)
            nc.scalar.activation(out=warm, in_=warm, func=AF.Exp)

            def proj_chunk(c):
                """QT/KT chunk c and V tiles for sequence chunk c."""
                for which, base in ((0, 0), (1, FEAT)):
                    sb = qT_sb if which == 0 else kT_sb
                    for ft in range(FT):
                        ps = mm_ps.tile([P, CH], f32, tag="mm", name="proj_ps")
                        for ko in range(KO):
                            nc.tensor.matmul(
                                ps,
                                lhsT=w_sb[:, ko, base + ft * P : base + (ft + 1) * P],
                                rhs=x_sb[:, ko, c * CH : (c + 1) * CH],
                                start=(ko == 0),
                                stop=(ko == KO - 1),
                            )
                        nc.vector.tensor_scalar_add(
                            out=sb[:, ft, c * CH : (c + 1) * CH],
                            in0=ps,
                            scalar1=bias_sb[:, which * FT + ft : which * FT + ft + 1],
                        )
                for sti in range(KPC):
                    st = c * KPC + sti
                    ps = mm_ps.tile([P, FEAT], f32, tag="mm", name="projv_ps")
                    for ko in range(KO):
                        nc.tensor.matmul(
                            ps,
                            lhsT=x_sb[:, ko, st * P : (st + 1) * P],
                            rhs=w_sb[:, ko, 2 * FEAT : 3 * FEAT],
                            start=(ko == 0),
                            stop=(ko == KO - 1),
                        )
                    nc.vector.tensor_add(
                        out=v_sb[:, st, :, 0:H],
                        in0=ps.rearrange("p (n h) -> p n h", h=H),
                        in1=bvb_sb.rearrange("p (n h) -> p n h", h=H),
                    )

            def attn_chunk(q):
                nkt = (q + 1) * KPC
                for hp in range(NH_LOC // 2):
                    ft = hp
                    h0, h1 = 2 * hp, 2 * hp + 1
                    zts = [
                        z_ps.tile([H + 1, CH], f32, tag="z", name="zt0"),
                        z_ps.tile([H + 1, CH], f32, tag="z", name="zt1"),
                    ]
                    for kt in range(nkt):
                        c = max(0, kt * P - q * CH)
                        pss = []
                        for hi, r0 in ((0, 0), (1, H)):
                            ps = sc_ps.tile([P, CH], f32, tag="sc", name="sc_ps_t")
                            nc.tensor.matmul(
                                ps[:, c:],
                                lhsT=kT_sb[r0 : r0 + H, ft, kt * P : (kt + 1) * P],
                                rhs=qT_sb[r0 : r0 + H, ft, q * CH + c : (q + 1) * CH],
                                start=True,
                                stop=True,
                            )
                            pss.append(ps)
                        pts = []
                        for hi in (0, 1):
                            pt = p_pool.tile([P, CH], bf16, tag="pt", name="pt_t")
                            nc.scalar.activation(
                                out=pt[:, c:], in_=pss[hi][:, c:], func=AF.Exp
                            )
                            if kt >= q * KPC:
                                # zero entries with sk > sq; in the trimmed
                                # frame: keep where f' - p >= 0
                                nc.gpsimd.affine_select(
                                    out=pt[:, c:],
                                    in_=pt[:, c:],
                                    compare_op=ALU.is_ge,
                                    fill=0.0,
                                    base=0,
                                    pattern=[[1, CH - c]],
                                    channel_multiplier=-1,
                                )
                            pts.append(pt)
                        for hi, h in ((0, h0), (1, h1)):
                            nc.tensor.matmul(
                                zts[hi][:, c:],
                                lhsT=v_sb[:, kt, h, :],
                                rhs=pts[hi][:, c:],
                                start=(kt == 0),
                                stop=(kt == nkt - 1),
                            )
                    # normalize rows 0..63 by row 64 (the denominator)
                    for hi, r0 in ((0, 0), (1, H)):
                        zt = zts[hi]
                        # custom-DVE ops read partition 0 regardless of the
                        # AP's base partition — stage the denominator row there
                        drow = n_pool.tile([1, CH], f32, tag="drow", name="drow_t")
                        nc.vector.tensor_copy(out=drow, in_=zt[H : H + 1, :])
                        rrow = n_pool.tile([1, CH], f32, tag="rrow", name="rrow_t")
                        nc.vector.reciprocal_approx_fast(out=rrow, in_=drow)
                        rb = n_pool.tile([H, CH], f32, tag="rb", name="rb_t")
                        nc.gpsimd.partition_broadcast(rb, rrow)
                        nc.vector.tensor_mul(
                            out=zn_sb[r0 : r0 + H, ft, q * CH : (q + 1) * CH],
                            in0=zt[0:H, :],
                            in1=rb,
                        )

            def out_chunk(q):
                # output projection for this chunk's sequence tiles;
                # dh pairs share the stationary zn tile per ft
                for sti in range(KPC):
                    st = q * KPC + sti
                    ysb = y_pool.tile([P, D], f32, tag="y", name="y_t")
                    psa = mm_ps.tile([P, D // 2], f32, tag="mm", name="yps_a")
                    psb = mm_ps.tile([P, D // 2], f32, tag="mm", name="yps_b")
                    for ft in range(FT):
                        for ps, dh in ((psa, 0), (psb, 1)):
                            nc.tensor.matmul(
                                ps,
                                lhsT=zn_sb[:, ft, st * P : (st + 1) * P],
                                rhs=wo_sb[:, ft, dh * (D // 2) : (dh + 1) * (D // 2)],
                                start=(ft == 0),
                                stop=(ft == FT - 1),
                            )
                    for ps, dh in ((psa, 0), (psb, 1)):
                        nc.vector.tensor_copy(
                            out=ysb[:, dh * (D // 2) : (dh + 1) * (D // 2)], in_=ps
                        )
                    nc.sync.dma_start(out=y[st * P : (st + 1) * P, :], in_=ysb)

            # interleave: projections for chunk c feed attention chunk c;
            # proj(c+1) is emitted before out(c) so the PE has independent
            # work while the last head pair of chunk c normalizes
            proj_chunk(0)
            for c in range(NCH):
                attn_chunk(c)
                if c + 1 < NCH:
                    proj_chunk(c + 1)
                out_chunk(c)

    nc.compile()
    return nc


def _get_nc():
    if "nc" not in _NC_CACHE:
        _NC_CACHE["nc"] = _build_nc()
    return _NC_CACHE["nc"]


def _shard_inputs(x, W_Q, W_K, W_V, W_O, b_Q, b_K, b_V):
    """Build the 8 per-core input maps. Core c -> (batch c % 4, head-group c // 4)."""
    in_maps = []
    scale = np.float32(1.0 / np.sqrt(H))
    for c in range(N_CORES):
        b = c % 4
        g = c // 4
        hs = slice(g * NH_LOC, (g + 1) * NH_LOC)
        xTb = np.ascontiguousarray(x[b].T).astype(BF16)
        wq = (W_Q[hs].transpose(1, 0, 2).reshape(D, FEAT) * scale).astype(BF16)
        wk = W_K[hs].transpose(1, 0, 2).reshape(D, FEAT).astype(BF16)
        wv = W_V[hs].transpose(1, 0, 2).reshape(D, FEAT).astype(BF16)
        wqkv = np.ascontiguousarray(np.concatenate([wq, wk, wv], axis=1))
        bqkv = np.concatenate(
            [
                (b_Q[hs].reshape(FEAT) * scale).astype(np.float32),
                b_K[hs].reshape(FEAT).astype(np.float32),
                b_V[hs].reshape(FEAT).astype(np.float32),
            ]
        )
        wob = np.ascontiguousarray(W_O[hs].reshape(FEAT, D)).astype(BF16)
        in_maps.append({"xT": xTb, "wqkv": wqkv, "bqkv": bqkv, "wo": wob})
    return in_maps


def _run(in_maps, trace=False):
    from concourse.bass_utils import run_bass_kernel_spmd

    nc = _get_nc()
    return run_bass_kernel_spmd(nc, in_maps, core_ids=list(range(N_CORES)), trace=trace)


def kernel(
    normalized_resid_pre,
    W_Q,
    W_K,
    W_V,
    W_O,
    b_Q,
    b_K,
    b_V,
    b_O,
):
    x = np.asarray(normalized_resid_pre, dtype=np.float32)
    in_maps = _shard_inputs(
        x,
        np.asarray(W_Q, np.float32),
        np.asarray(W_K, np.float32),
        np.asarray(W_V, np.float32),
        np.asarray(W_O, np.float32),
        np.asarray(b_Q, np.float32),
        np.asarray(b_K, np.float32),
        np.asarray(b_V, np.float32),
    )
    res = _run(in_maps)
    bO = np.asarray(b_O, np.float32)
    out = np.empty((4, S, D), dtype=np.float32)
    for b in range(4):
        out[b] = res.results[b]["y"] + res.results[4 + b]["y"] + bO
    return out
